# revision 33
# baseline (speedup 1.0000x reference)
"""HNet energy-via-edge-matching kernel for 8 Trainium2 NeuronCores.

Math (matches the reference exactly, in exact integer arithmetic):
  temp[i,e] = 2*na[i, idx0[e]] + na[i, idx1[e]]          in {0,1,2,3}
  es = code[temp], code = [NOR=2, NCONV=3, NIMPL=5, AND=9]
  filter keeps es values in edge_type_filter, else NULL=0
  energies[i,j] = #{e: L[j,e]==es'[i,e] or L[j,e]==0}
               = null_count[j] + sum_{v kept} (temp==tmap[v]) . (L==v)
  output = energies - min(energies)

Device decomposition per core (4 point-groups x 2 cmp-groups), v4:
  The kernel is a pair of one-hot popcount GEMMs over K=n_edges per kept
  edge type.  Operand planes are O(input-size) preprocessing and are
  staged host-side (like the null counts):
    A_v[e,i] = (temp[i,e]==tmap[v])   fp8, edge-major, per point-group
    B_v[e,j] = (L[j,e]==v)            fp8, edge-major, per cmp-group
  Device: energies accumulate j-partitioned, psum[j,i] over the full K
  per bank (kk-outer loop), fp8 DoubleRow matmuls with lhsT=B chunk,
  rhs=A chunk (one matmul per 256-edge slice per j-subtile per type).
  null_count[j] is added as a per-partition ACT bias during the
  PSUM->SBUF copy (Relu == identity on these non-negative counts);
  per-tile min reduced on DVE.  B streams through a rotating pool; A is
  SBUF-resident (reused by all 16 output tiles).
Host only: input staging/layout (operand planes, null counts, tiling),
  global min of per-core mins, final subtract and transpose during
  unshard (elementwise/layout, exact fp32).
"""

import numpy as np
import ml_dtypes

import concourse.bacc as bacc
import concourse.mybir as mybir
from concourse.tile import TileContext
from concourse.bass_utils import run_bass_kernel_spmd

# ---- problem constants (hardcoded from spec) ----
N_PTS, N_NODES, N_EDGES, N_CMP = 2048, 1024, 8192, 4096
PGROUPS, CGROUPS = 4, 2          # 8 cores = 4 point-groups x 2 cmp-groups
P = N_PTS // PGROUPS             # 512 points per core
C = N_CMP // CGROUPS             # 2048 cmp columns per core
ECHUNKS = N_EDGES // 128         # 64 edge chunks of 128
NTILES = C // 512                # 4 cmp tiles of 512 per core
JTILES = 4                       # 4 j-subtiles of 128 per cmp tile
BBLK = 8                         # edge chunks per steady-state stream tile
NBBLK = ECHUNKS // BBLK          # 8 tiles per (plane, cmp tile)
# block schedule (start_chunk, n_chunks): small first blocks so the first
# matmuls only wait on a few hundred KB of DMA
BLOCKS = [(0, 2), (2, 2), (4, 4)] + [(8 + 8 * k, 8) for k in range(7)]
# cmp-tile schedule (j_start, j_width): the first tile is double-wide so
# the ramp reuses each fresh A chunk across 8 psum banks (the DMA feed is
# the binding constraint until the A planes are fully resident)
CTILES = [(0, 1024), (1024, 512), (1536, 512)]

FP8 = mybir.dt.float8e4
F32 = mybir.dt.float32
NP_FP8 = ml_dtypes.float8_e4m3
DR = mybir.MatmulPerfMode.DoubleRow
RELU = mybir.ActivationFunctionType.Relu

_CODE2TEMP = {2: 0, 3: 1, 5: 2, 9: 3}   # EDG code value -> temp index

_nc_cache: dict = {}


def _build_nc(npair):
    """Build the SPMD Bass program for `npair` kept edge types."""
    nc = bacc.Bacc(None)

    # pre-tiled inputs (host lays out so every DMA is per-partition dense):
    # The kept-pair dim is an OUTER free dim so ONE dma_start (~600ns of
    # descriptor generation on the issuing engine) carries all planes of
    # a block while K-pair rows stay 512B-contiguous for the PE streams:
    #   Am  : [128, npair, ECHUNKS*512]
    #         [ki, q, ec*512+i] = (temp[pg*P+i, ec*128+ki]==tmap[v_q])
    #   Bm  : [C//512, 128, npair, ECHUNKS*512]   (512-wide j groups)
    #         [jg, ki, q, ec*512+j] = (L[cg*C+jg*512+j, ec*128+ki]==v_q)
    #   nulc: [128, NTILES*JTILES]  [jj, nt*4+jt] = null_count[j] (f32)
    Am = nc.dram_tensor("Am", [128, npair, ECHUNKS * 512], FP8,
                        kind="ExternalInput")
    Bm = nc.dram_tensor("Bm", [C // 512, 128, npair, ECHUNKS * 512], FP8,
                        kind="ExternalInput")
    nulc = nc.dram_tensor("nulc", [128, NTILES * JTILES], F32,
                          kind="ExternalInput")
    # outputs: en is j-major [C, P]; host transposes during unshard
    en = nc.dram_tensor("en", [C, P], F32, kind="ExternalOutput")
    mins = nc.dram_tensor("mins", [128, NTILES * JTILES], F32,
                          kind="ExternalOutput")

    with TileContext(nc) as tc:
        with (
            tc.tile_pool(name="const", bufs=1) as const_pool,
            tc.tile_pool(name="amask", bufs=1) as a_pool,
            tc.tile_pool(name="bstream", bufs=12) as b_pool,
            tc.tile_pool(name="out", bufs=8) as out_pool,
            tc.tile_pool(name="psum", bufs=8, space="PSUM") as psum_pool,
        ):
            # nulc + A-plane DMAs issue from the (otherwise idle) ACT
            # hwdge queue, in parallel with the B stream's sync-queue
            # issues — descriptor generation is ~600ns per dma_start on
            # the issuing engine and gates the ramp otherwise
            nulc_sb = const_pool.tile([128, NTILES * JTILES], F32, tag="nulc")
            nc.scalar.dma_start(out=nulc_sb[:], in_=nulc[:])
            mins_sb = const_pool.tile([128, NTILES * JTILES], F32, tag="mins")

            # PE pstate warm-up: dependency-free dummy matmuls keep the
            # tensor engine busy while the first operand DMAs land, so the
            # real stream starts at full clock
            warm_w = const_pool.tile([128, 2, 128], FP8, tag="warmw")
            nc.vector.memset(warm_w[:], 0.0)
            warm_r = const_pool.tile([128, 2, 512], FP8, tag="warmr")
            nc.vector.memset(warm_r[:], 0.0)
            wps = [psum_pool.tile([128, 512], F32, name=f"wp{k}",
                                  tag="ps") for k in range(4)]
            for k in range(12):
                nc.tensor.matmul(wps[k % 4], lhsT=warm_w[:], rhs=warm_r[:],
                                 start=True, stop=True, perf_mode=DR,
                                 skip_group_check=True)

            a_sb = a_pool.tile([128, npair, ECHUNKS, 512], FP8, tag="a")

            def b_dma(jst, jw, st, n):
                # one [128, npair, n, 512] tile per 512-wide j half
                bts = []
                for jh in range(jw // 512):
                    bt = b_pool.tile([128, npair, n, 512], FP8, name="bt",
                                     tag="b")
                    jg = jst // 512 + jh
                    nc.sync.dma_start(
                        out=bt[:],
                        in_=Bm[jg, :, :, st * 512:(st + n) * 512])
                    bts.append(bt)
                return bts

            b_first = []
            jst0, jw0 = CTILES[0]
            for (st, n) in BLOCKS:
                nc.scalar.dma_start(
                    out=a_sb[:, :, st:st + n, :],
                    in_=Am[:, :, st * 512:(st + n) * 512])
                b_first.append(b_dma(jst0, jw0, st, n))

            def drain(eps, jst, jt):
                idx = jst // 128 + jt
                ot = out_pool.tile([128, P], F32, name="ot", tag="out")
                # Relu is an exact identity here: gemm counts and null
                # counts are both non-negative
                nc.scalar.activation(ot[:], eps[jt], RELU,
                                     bias=nulc_sb[:, idx:idx + 1],
                                     scale=1.0)
                nc.scalar.dma_start(
                    out=en[idx * 128:(idx + 1) * 128, :], in_=ot[:])
                nc.vector.tensor_reduce(
                    out=mins_sb[:, idx:idx + 1], in_=ot[:],
                    axis=mybir.AxisListType.X, op=mybir.AluOpType.min)

            for ti, (jst, jw) in enumerate(CTILES):
                njt = jw // 128
                eps = [psum_pool.tile([128, P], F32, name=f"ep{jt}",
                                      tag="ps") for jt in range(njt)]
                for bi, (st, n) in enumerate(BLOCKS):
                    bts = b_first[bi] if ti == 0 else b_dma(jst, jw, st, n)
                    last_blk = bi == len(BLOCKS) - 1
                    for jt in range(njt):
                        for kk in range(n // 2):
                            ec2 = st + 2 * kk
                            for q in range(npair):
                                first = bi == 0 and kk == 0 and q == 0
                                last = (last_blk and kk == n // 2 - 1
                                        and q == npair - 1)
                                nc.tensor.matmul(
                                    eps[jt],
                                    lhsT=bts[jt // 4][
                                        :, q, 2 * kk:2 * kk + 2,
                                        (jt % 4) * 128:(jt % 4 + 1) * 128],
                                    rhs=a_sb[:, q, ec2:ec2 + 2, :],
                                    start=first, stop=last,
                                    perf_mode=DR, skip_group_check=True)
                        if last_blk:
                            # drain each bank right after its last matmul
                            drain(eps, jst, jt)
            nc.sync.dma_start(out=mins[:], in_=mins_sb[:])
    if not nc.is_finalized():
        nc.finalize()
    return nc


def _get_nc(npair):
    if npair not in _nc_cache:
        _nc_cache[npair] = _build_nc(npair)
    return _nc_cache[npair]


def _prep_inputs(node_activations, learned_edge_states, edge_endnode_idx,
                 pairs):
    na = np.asarray(node_activations)
    L = np.asarray(learned_edge_states, dtype=np.float32)
    idx = np.asarray(edge_endnode_idx)

    temp = (na[:, idx[:, 0]] * 2 + na[:, idx[:, 1]]).T   # [edges, pts] int
    LT = L.T                                             # [edges, cmp]
    null_count = (L == 0.0).sum(axis=1).astype(np.float32)   # [cmp]

    ams = []
    for pg in range(PGROUPS):
        t = temp[:, pg * P:(pg + 1) * P]
        # [ki, q, ec*512+i]
        am = np.stack([(t == tv).reshape(ECHUNKS, 128, P)
                       .transpose(1, 0, 2).reshape(128, -1)
                       for tv, _lv in pairs], axis=1)
        ams.append(np.ascontiguousarray(am).astype(NP_FP8))
    bms = []
    for cg in range(CGROUPS):
        Lc = LT[:, cg * C:(cg + 1) * C]                  # [edges, C]
        # [jg, ki, q, ec*512+j]
        bm = np.stack([(Lc == float(lv))
                       .reshape(ECHUNKS, 128, C // 512, 512)
                       .transpose(2, 1, 0, 3)
                       .reshape(C // 512, 128, -1)
                       for _tv, lv in pairs], axis=2)
        bm = np.ascontiguousarray(bm).astype(NP_FP8)
        nulc = np.ascontiguousarray(
            null_count[cg * C:(cg + 1) * C]
            .reshape(NTILES * JTILES, 128).T).astype(np.float32)
        bms.append((bm, nulc))

    in_maps = []
    for pg in range(PGROUPS):
        for cg in range(CGROUPS):
            in_maps.append({
                "Am": ams[pg],
                "Bm": bms[cg][0],
                "nulc": bms[cg][1],
            })
    return in_maps


def _kept_pairs(edge_type_filter):
    seen = []
    for v in np.asarray(edge_type_filter).ravel().tolist():
        v = int(v)
        if v in _CODE2TEMP and v not in [p[1] for p in seen]:
            seen.append((_CODE2TEMP[v], v))
    return tuple(seen)


def kernel(node_activations, learned_edge_states, edge_endnode_idx,
           edge_type_filter, _trace=False, _tmpdir=None):
    pairs = _kept_pairs(edge_type_filter)
    L = np.asarray(learned_edge_states, dtype=np.float32)
    if len(pairs) == 0:
        # nothing kept: energies are null_count rows broadcast
        null_count = (L == 0.0).sum(axis=1).astype(np.float32)
        en = np.broadcast_to(null_count[None, :], (N_PTS, N_CMP)).copy()
        return en - en.min()

    nc = _get_nc(len(pairs))
    in_maps = _prep_inputs(node_activations, learned_edge_states,
                           edge_endnode_idx, pairs)
    res = run_bass_kernel_spmd(nc, in_maps, core_ids=list(range(8)),
                               trace=_trace, tmpdir=_tmpdir)
    out = np.empty((N_PTS, N_CMP), dtype=np.float32)
    gmin = np.inf
    for ci in range(8):
        pg, cg = ci // CGROUPS, ci % CGROUPS
        r = res.results[ci]
        out[pg * P:(pg + 1) * P, cg * C:(cg + 1) * C] = r["en"].T
        gmin = min(gmin, float(r["mins"].min()))
    out -= np.float32(gmin)
    if _trace:
        kernel._last_results = res
    return out


# revision 36
# speedup vs baseline: 1.2250x; 1.2250x over previous
"""HNet energy-via-edge-matching kernel for 8 Trainium2 NeuronCores.

Math (matches the reference exactly, in exact integer arithmetic):
  temp[i,e] = 2*na[i, idx0[e]] + na[i, idx1[e]]          in {0,1,2,3}
  es = code[temp], code = [NOR=2, NCONV=3, NIMPL=5, AND=9]
  filter keeps es values in edge_type_filter, else NULL=0
  energies[i,j] = #{e: L[j,e]==es'[i,e] or L[j,e]==0}
               = null_count[j] + sum_{v kept} (temp==tmap[v]) . (L==v)
  output = energies - min(energies)

Device decomposition per core (4 point-groups x 2 cmp-groups), v4:
  The kernel is a pair of one-hot popcount GEMMs over K=n_edges per kept
  edge type.  Operand planes are O(input-size) preprocessing and are
  staged host-side (like the null counts):
    A_v[e,i] = (temp[i,e]==tmap[v])   fp8, edge-major, per point-group
    B_v[e,j] = (L[j,e]==v)            fp8, edge-major, per cmp-group
  Device: energies accumulate j-partitioned, psum[j,i] over the full K
  per bank (kk-outer loop), fp8 DoubleRow matmuls with lhsT=B chunk,
  rhs=A chunk (one matmul per 256-edge slice per j-subtile per type).
  null_count[j] is added as a per-partition ACT bias during the
  PSUM->SBUF copy (Relu == identity on these non-negative counts);
  per-tile min reduced on DVE.  B streams through a rotating pool; A is
  SBUF-resident (reused by all 16 output tiles).
Host only: input staging/layout (operand planes, null counts, tiling),
  global min of per-core mins, final subtract and transpose during
  unshard (elementwise/layout, exact fp32).
"""

import numpy as np
import ml_dtypes

import concourse.bacc as bacc
import concourse.mybir as mybir
from concourse.tile import TileContext
from concourse.bass_utils import run_bass_kernel_spmd

# ---- problem constants (hardcoded from spec) ----
N_PTS, N_NODES, N_EDGES, N_CMP = 2048, 1024, 8192, 4096
PGROUPS, CGROUPS = 4, 2          # 8 cores = 4 point-groups x 2 cmp-groups
P = N_PTS // PGROUPS             # 512 points per core
C = N_CMP // CGROUPS             # 2048 cmp columns per core
ECHUNKS = N_EDGES // 128         # 64 edge chunks of 128
NTILES = C // 512                # 4 cmp tiles of 512 per core
JTILES = 4                       # 4 j-subtiles of 128 per cmp tile
BBLK = 8                         # edge chunks per steady-state stream tile
NBBLK = ECHUNKS // BBLK          # 8 tiles per (plane, cmp tile)
# block schedule (start_chunk, n_chunks): small first blocks so the first
# matmuls only wait on a few hundred KB of DMA
BLOCKS = [(0, 2), (2, 2), (4, 4)] + [(8 + 8 * k, 8) for k in range(7)]
# cmp-tile schedule (j_start, j_width): the first tile is double-wide so
# the ramp reuses each fresh A chunk across 8 psum banks (the DMA feed is
# the binding constraint until the A planes are fully resident)
CTILES = [(0, 1024), (1024, 512), (1536, 512)]

FP8 = mybir.dt.float8e4
F32 = mybir.dt.float32
NP_FP8 = ml_dtypes.float8_e4m3
DR = mybir.MatmulPerfMode.DoubleRow
RELU = mybir.ActivationFunctionType.Relu

_CODE2TEMP = {2: 0, 3: 1, 5: 2, 9: 3}   # EDG code value -> temp index

_nc_cache: dict = {}


def _build_nc(npair):
    """Build the SPMD Bass program for `npair` kept edge types."""
    nc = bacc.Bacc(None)

    # pre-tiled inputs (host lays out so every DMA is per-partition dense):
    #   Am  : [npair, 128, ECHUNKS*512]
    #         [q, ki, ec*512+i] = (temp[pg*P+i, ec*128+ki]==tmap[v_q])
    #   Bm  : [npair, C//512, 128, ECHUNKS*512]  (512-wide j groups, so
    #         every stream DMA reads contiguous multi-KB partition lines)
    #         [q, jg, ki, ec*512+j] = (L[cg*C+jg*512+j, ec*128+ki]==v_q)
    #   nulc: [128, NTILES*JTILES]  [jj, nt*4+jt] = null_count[j] (f32)
    Am = nc.dram_tensor("Am", [npair, 128, ECHUNKS * 512], FP8,
                        kind="ExternalInput")
    Bm = nc.dram_tensor("Bm", [npair, C // 512, 128, ECHUNKS * 512], FP8,
                        kind="ExternalInput")
    nulc = nc.dram_tensor("nulc", [128, NTILES * JTILES], F32,
                          kind="ExternalInput")
    # outputs: en is j-major [C, P]; host transposes during unshard
    en = nc.dram_tensor("en", [C, P], F32, kind="ExternalOutput")
    mins = nc.dram_tensor("mins", [128, NTILES * JTILES], F32,
                          kind="ExternalOutput")

    with TileContext(nc) as tc:
        with (
            tc.tile_pool(name="const", bufs=1) as const_pool,
            tc.tile_pool(name="amask", bufs=1) as a_pool,
            tc.tile_pool(name="bstream", bufs=24) as b_pool,
            tc.tile_pool(name="out", bufs=8) as out_pool,
            tc.tile_pool(name="psum", bufs=8, space="PSUM") as psum_pool,
        ):
            nulc_sb = const_pool.tile([128, NTILES * JTILES], F32, tag="nulc")
            nc.scalar.dma_start(out=nulc_sb[:], in_=nulc[:])
            mins_sb = const_pool.tile([128, NTILES * JTILES], F32, tag="mins")

            # PE pstate warm-up: dependency-free dummy matmuls keep the
            # tensor engine busy while the first operand DMAs land, so the
            # real stream starts at full clock (cold-pstate dummies run
            # ~2x slower than steady, hence the short chain)
            warm_w = const_pool.tile([128, 2, 128], FP8, tag="warmw")
            nc.vector.memset(warm_w[:], 0.0)
            warm_r = const_pool.tile([128, 2, 512], FP8, tag="warmr")
            nc.vector.memset(warm_r[:], 0.0)
            wps = [psum_pool.tile([128, 512], F32, name=f"wp{k}",
                                  tag="ps") for k in range(4)]
            for k in range(12):
                nc.tensor.matmul(wps[k % 4], lhsT=warm_w[:], rhs=warm_r[:],
                                 start=True, stop=True, perf_mode=DR,
                                 skip_group_check=True)

            # A planes resident; block DMAs interleaved with the first cmp
            # tile's B stream so the ramp is supply-matched
            a_tiles = [a_pool.tile([128, ECHUNKS, P], FP8, name=f"a{q}",
                                   tag=f"a{q}") for q in range(npair)]

            def b_dma(jst, jw, st, n):
                # one [128, n, 512] tile per (pair, 512-wide j half)
                bts = []
                for q in range(npair):
                    row = []
                    for jh in range(jw // 512):
                        bt = b_pool.tile([128, n, 512], FP8, name="bt",
                                         tag="b")
                        jg = jst // 512 + jh
                        nc.sync.dma_start(
                            out=bt[:],
                            in_=Bm[q, jg, :, st * 512:(st + n) * 512])
                        row.append(bt)
                    bts.append(row)
                return bts

            b_first = []
            jst0, jw0 = CTILES[0]
            for (st, n) in BLOCKS:
                for q in range(npair):
                    nc.scalar.dma_start(
                        out=a_tiles[q][:, st:st + n, :],
                        in_=Am[q, :, st * 512:(st + n) * 512])
                b_first.append(b_dma(jst0, jw0, st, n))

            def drain(eps, jst, jt):
                idx = jst // 128 + jt
                ot = out_pool.tile([128, P], F32, name="ot", tag="out")
                # Relu is an exact identity here: gemm counts and null
                # counts are both non-negative.  The en DMA issues from
                # ACT (data-local, keeps the ~600ns descriptor cost off
                # the sync queue's tail)
                nc.scalar.activation(ot[:], eps[jt], RELU,
                                     bias=nulc_sb[:, idx:idx + 1],
                                     scale=1.0)
                nc.scalar.dma_start(
                    out=en[idx * 128:(idx + 1) * 128, :], in_=ot[:])
                nc.vector.tensor_reduce(
                    out=mins_sb[:, idx:idx + 1], in_=ot[:],
                    axis=mybir.AxisListType.X, op=mybir.AluOpType.min)

            for ti, (jst, jw) in enumerate(CTILES):
                njt = jw // 128
                eps = [psum_pool.tile([128, P], F32, name=f"ep{jt}",
                                      tag="ps") for jt in range(njt)]
                for bi, (st, n) in enumerate(BLOCKS):
                    bts = b_first[bi] if ti == 0 else b_dma(jst, jw, st, n)
                    last_blk = bi == len(BLOCKS) - 1
                    for jt in range(njt):
                        for kk in range(n // 2):
                            ec2 = st + 2 * kk
                            for q in range(npair):
                                first = bi == 0 and kk == 0 and q == 0
                                last = (last_blk and kk == n // 2 - 1
                                        and q == npair - 1)
                                nc.tensor.matmul(
                                    eps[jt],
                                    lhsT=bts[q][jt // 4][
                                        :, 2 * kk:2 * kk + 2,
                                        (jt % 4) * 128:(jt % 4 + 1) * 128],
                                    rhs=a_tiles[q][:, ec2:ec2 + 2, :],
                                    start=first, stop=last,
                                    perf_mode=DR, skip_group_check=True)
                        if last_blk:
                            # drain each bank right after its last matmul
                            drain(eps, jst, jt)
            nc.sync.dma_start(out=mins[:], in_=mins_sb[:])
    if not nc.is_finalized():
        nc.finalize()
    return nc


def _get_nc(npair):
    if npair not in _nc_cache:
        _nc_cache[npair] = _build_nc(npair)
    return _nc_cache[npair]


def _tile_plane(p):
    """[rows(edges), cols] bool -> [128, ECHUNKS*cols] fp8 (ki-major)."""
    cols = p.shape[1]
    return np.ascontiguousarray(
        p.reshape(ECHUNKS, 128, cols).transpose(1, 0, 2)
        .reshape(128, ECHUNKS * cols)).astype(NP_FP8)


def _prep_inputs(node_activations, learned_edge_states, edge_endnode_idx,
                 pairs):
    na = np.asarray(node_activations)
    L = np.asarray(learned_edge_states, dtype=np.float32)
    idx = np.asarray(edge_endnode_idx)

    temp = (na[:, idx[:, 0]] * 2 + na[:, idx[:, 1]]).T   # [edges, pts] int
    LT = L.T                                             # [edges, cmp]
    null_count = (L == 0.0).sum(axis=1).astype(np.float32)   # [cmp]

    ams = []
    for pg in range(PGROUPS):
        t = temp[:, pg * P:(pg + 1) * P]
        ams.append(np.stack([_tile_plane(t == tv) for tv, _lv in pairs]))
    bms = []
    for cg in range(CGROUPS):
        Lc = LT[:, cg * C:(cg + 1) * C]                  # [edges, C]
        bm = np.stack([
            np.ascontiguousarray(
                (Lc == float(lv)).reshape(ECHUNKS, 128, C // 512, 512)
                .transpose(2, 1, 0, 3)
                .reshape(C // 512, 128, ECHUNKS * 512)).astype(NP_FP8)
            for _tv, lv in pairs])
        nulc = np.ascontiguousarray(
            null_count[cg * C:(cg + 1) * C]
            .reshape(NTILES * JTILES, 128).T).astype(np.float32)
        bms.append((np.ascontiguousarray(bm), nulc))

    in_maps = []
    for pg in range(PGROUPS):
        for cg in range(CGROUPS):
            in_maps.append({
                "Am": ams[pg],
                "Bm": bms[cg][0],
                "nulc": bms[cg][1],
            })
    return in_maps


def _kept_pairs(edge_type_filter):
    seen = []
    for v in np.asarray(edge_type_filter).ravel().tolist():
        v = int(v)
        if v in _CODE2TEMP and v not in [p[1] for p in seen]:
            seen.append((_CODE2TEMP[v], v))
    return tuple(seen)


def kernel(node_activations, learned_edge_states, edge_endnode_idx,
           edge_type_filter, _trace=False, _tmpdir=None):
    pairs = _kept_pairs(edge_type_filter)
    L = np.asarray(learned_edge_states, dtype=np.float32)
    if len(pairs) == 0:
        # nothing kept: energies are null_count rows broadcast
        null_count = (L == 0.0).sum(axis=1).astype(np.float32)
        en = np.broadcast_to(null_count[None, :], (N_PTS, N_CMP)).copy()
        return en - en.min()

    nc = _get_nc(len(pairs))
    in_maps = _prep_inputs(node_activations, learned_edge_states,
                           edge_endnode_idx, pairs)
    res = run_bass_kernel_spmd(nc, in_maps, core_ids=list(range(8)),
                               trace=_trace, tmpdir=_tmpdir)
    out = np.empty((N_PTS, N_CMP), dtype=np.float32)
    gmin = np.inf
    for ci in range(8):
        pg, cg = ci // CGROUPS, ci % CGROUPS
        r = res.results[ci]
        out[pg * P:(pg + 1) * P, cg * C:(cg + 1) * C] = r["en"].T
        gmin = min(gmin, float(r["mins"].min()))
    out -= np.float32(gmin)
    if _trace:
        kernel._last_results = res
    return out


# revision 37
# speedup vs baseline: 1.2297x; 1.0039x over previous
"""HNet energy-via-edge-matching kernel for 8 Trainium2 NeuronCores.

Math (matches the reference exactly, in exact integer arithmetic):
  temp[i,e] = 2*na[i, idx0[e]] + na[i, idx1[e]]          in {0,1,2,3}
  es = code[temp], code = [NOR=2, NCONV=3, NIMPL=5, AND=9]
  filter keeps es values in edge_type_filter, else NULL=0
  energies[i,j] = #{e: L[j,e]==es'[i,e] or L[j,e]==0}
               = null_count[j] + sum_{v kept} (temp==tmap[v]) . (L==v)
  output = energies - min(energies)

Device decomposition per core (4 point-groups x 2 cmp-groups), v4:
  The kernel is a pair of one-hot popcount GEMMs over K=n_edges per kept
  edge type.  Operand planes are O(input-size) preprocessing and are
  staged host-side (like the null counts):
    A_v[e,i] = (temp[i,e]==tmap[v])   fp8, edge-major, per point-group
    B_v[e,j] = (L[j,e]==v)            fp8, edge-major, per cmp-group
  Device: energies accumulate j-partitioned, psum[j,i] over the full K
  per bank (kk-outer loop), fp8 DoubleRow matmuls with lhsT=B chunk,
  rhs=A chunk (one matmul per 256-edge slice per j-subtile per type).
  null_count[j] is added as a per-partition ACT bias during the
  PSUM->SBUF copy (Relu == identity on these non-negative counts);
  per-tile min reduced on DVE.  B streams through a rotating pool; A is
  SBUF-resident (reused by all 16 output tiles).
Host only: input staging/layout (operand planes, null counts, tiling),
  global min of per-core mins, final subtract and transpose during
  unshard (elementwise/layout, exact fp32).
"""

import numpy as np
import ml_dtypes

import concourse.bacc as bacc
import concourse.mybir as mybir
from concourse.tile import TileContext
from concourse.bass_utils import run_bass_kernel_spmd

# ---- problem constants (hardcoded from spec) ----
N_PTS, N_NODES, N_EDGES, N_CMP = 2048, 1024, 8192, 4096
PGROUPS, CGROUPS = 4, 2          # 8 cores = 4 point-groups x 2 cmp-groups
P = N_PTS // PGROUPS             # 512 points per core
C = N_CMP // CGROUPS             # 2048 cmp columns per core
ECHUNKS = N_EDGES // 128         # 64 edge chunks of 128
NTILES = C // 512                # 4 cmp tiles of 512 per core
JTILES = 4                       # 4 j-subtiles of 128 per cmp tile
BBLK = 8                         # edge chunks per steady-state stream tile
NBBLK = ECHUNKS // BBLK          # 8 tiles per (plane, cmp tile)
# block schedule (start_chunk, n_chunks): small first blocks so the first
# matmuls only wait on a few hundred KB of DMA
BLOCKS = [(0, 2), (2, 2), (4, 4)] + [(8 + 8 * k, 8) for k in range(7)]
# cmp-tile schedule (j_start, j_width): the first tile is double-wide so
# the ramp reuses each fresh A chunk across 8 psum banks (the DMA feed is
# the binding constraint until the A planes are fully resident)
CTILES = [(0, 1024), (1024, 512), (1536, 512)]

FP8 = mybir.dt.float8e4
F32 = mybir.dt.float32
NP_FP8 = ml_dtypes.float8_e4m3
DR = mybir.MatmulPerfMode.DoubleRow
RELU = mybir.ActivationFunctionType.Relu

_CODE2TEMP = {2: 0, 3: 1, 5: 2, 9: 3}   # EDG code value -> temp index

_nc_cache: dict = {}


def _build_nc(npair):
    """Build the SPMD Bass program for `npair` kept edge types."""
    nc = bacc.Bacc(None)

    # pre-tiled inputs (host lays out so every DMA is per-partition dense):
    #   Am  : [npair, 128, ECHUNKS*512]
    #         [q, ki, ec*512+i] = (temp[pg*P+i, ec*128+ki]==tmap[v_q])
    #   Bm  : [npair, C//512, 128, ECHUNKS*512]  (512-wide j groups, so
    #         every stream DMA reads contiguous multi-KB partition lines)
    #         [q, jg, ki, ec*512+j] = (L[cg*C+jg*512+j, ec*128+ki]==v_q)
    #   nulc: [128, NTILES*JTILES]  [jj, nt*4+jt] = null_count[j] (f32)
    Am = nc.dram_tensor("Am", [npair, 128, ECHUNKS * 512], FP8,
                        kind="ExternalInput")
    Bm = nc.dram_tensor("Bm", [npair, C // 512, 128, ECHUNKS * 512], FP8,
                        kind="ExternalInput")
    nulc = nc.dram_tensor("nulc", [128, NTILES * JTILES], F32,
                          kind="ExternalInput")
    # outputs: en is j-major [C, P]; host transposes during unshard
    en = nc.dram_tensor("en", [C, P], F32, kind="ExternalOutput")
    mins = nc.dram_tensor("mins", [128, NTILES * JTILES], F32,
                          kind="ExternalOutput")

    with TileContext(nc) as tc:
        with (
            tc.tile_pool(name="const", bufs=1) as const_pool,
            tc.tile_pool(name="amask", bufs=1) as a_pool,
            tc.tile_pool(name="bstream", bufs=24) as b_pool,
            tc.tile_pool(name="out", bufs=8) as out_pool,
            tc.tile_pool(name="psum", bufs=8, space="PSUM") as psum_pool,
        ):
            nulc_sb = const_pool.tile([128, NTILES * JTILES], F32, tag="nulc")
            nc.sync.dma_start(out=nulc_sb[:], in_=nulc[:])
            mins_sb = const_pool.tile([128, NTILES * JTILES], F32, tag="mins")

            # A planes resident; block DMAs interleaved with the first cmp
            # tile's B stream so the ramp is supply-matched
            a_tiles = [a_pool.tile([128, ECHUNKS, P], FP8, name=f"a{q}",
                                   tag=f"a{q}") for q in range(npair)]

            def b_dma(jst, jw, st, n):
                # one [128, n, 512] tile per (pair, 512-wide j half)
                bts = []
                for q in range(npair):
                    row = []
                    for jh in range(jw // 512):
                        bt = b_pool.tile([128, n, 512], FP8, name="bt",
                                         tag="b")
                        jg = jst // 512 + jh
                        nc.sync.dma_start(
                            out=bt[:],
                            in_=Bm[q, jg, :, st * 512:(st + n) * 512])
                        row.append(bt)
                    bts.append(row)
                return bts

            b_first = []
            jst0, jw0 = CTILES[0]
            for (st, n) in BLOCKS:
                for q in range(npair):
                    nc.sync.dma_start(
                        out=a_tiles[q][:, st:st + n, :],
                        in_=Am[q, :, st * 512:(st + n) * 512])
                b_first.append(b_dma(jst0, jw0, st, n))

            def drain(eps, jst, jt, nsplit=1):
                idx = jst // 128 + jt
                ot = out_pool.tile([128, P], F32, name="ot", tag="out")
                w = P // nsplit
                for h in range(nsplit):
                    sl = slice(h * w, (h + 1) * w)
                    # Relu is an exact identity here: gemm counts and
                    # null counts are both non-negative
                    nc.scalar.activation(ot[:, sl], eps[jt][:, sl], RELU,
                                         bias=nulc_sb[:, idx:idx + 1],
                                         scale=1.0)
                    nc.sync.dma_start(
                        out=en[idx * 128:(idx + 1) * 128, sl],
                        in_=ot[:, sl])
                nc.vector.tensor_reduce(
                    out=mins_sb[:, idx:idx + 1], in_=ot[:],
                    axis=mybir.AxisListType.X, op=mybir.AluOpType.min)

            for ti, (jst, jw) in enumerate(CTILES):
                njt = jw // 128
                eps = [psum_pool.tile([128, P], F32, name=f"ep{jt}",
                                      tag="ps") for jt in range(njt)]
                for bi, (st, n) in enumerate(BLOCKS):
                    bts = b_first[bi] if ti == 0 else b_dma(jst, jw, st, n)
                    last_blk = bi == len(BLOCKS) - 1
                    very_last = ti == len(CTILES) - 1 and last_blk
                    for jt in range(njt):
                        for kk in range(n // 2):
                            ec2 = st + 2 * kk
                            for q in range(npair):
                                first = bi == 0 and kk == 0 and q == 0
                                last = (last_blk and kk == n // 2 - 1
                                        and q == npair - 1)
                                nc.tensor.matmul(
                                    eps[jt],
                                    lhsT=bts[q][jt // 4][
                                        :, 2 * kk:2 * kk + 2,
                                        (jt % 4) * 128:(jt % 4 + 1) * 128],
                                    rhs=a_tiles[q][:, ec2:ec2 + 2, :],
                                    start=first, stop=last,
                                    perf_mode=DR, skip_group_check=True)
                        if last_blk:
                            # drain each bank right after its last matmul;
                            # split the very last one so copy/DMA pipeline
                            drain(eps, jst, jt,
                                  nsplit=4 if very_last and jt == njt - 1
                                  else 1)
            nc.sync.dma_start(out=mins[:], in_=mins_sb[:])
    if not nc.is_finalized():
        nc.finalize()
    return nc


def _get_nc(npair):
    if npair not in _nc_cache:
        _nc_cache[npair] = _build_nc(npair)
    return _nc_cache[npair]


def _tile_plane(p):
    """[rows(edges), cols] bool -> [128, ECHUNKS*cols] fp8 (ki-major)."""
    cols = p.shape[1]
    return np.ascontiguousarray(
        p.reshape(ECHUNKS, 128, cols).transpose(1, 0, 2)
        .reshape(128, ECHUNKS * cols)).astype(NP_FP8)


def _prep_inputs(node_activations, learned_edge_states, edge_endnode_idx,
                 pairs):
    na = np.asarray(node_activations)
    L = np.asarray(learned_edge_states, dtype=np.float32)
    idx = np.asarray(edge_endnode_idx)

    temp = (na[:, idx[:, 0]] * 2 + na[:, idx[:, 1]]).T   # [edges, pts] int
    LT = L.T                                             # [edges, cmp]
    null_count = (L == 0.0).sum(axis=1).astype(np.float32)   # [cmp]

    ams = []
    for pg in range(PGROUPS):
        t = temp[:, pg * P:(pg + 1) * P]
        ams.append(np.stack([_tile_plane(t == tv) for tv, _lv in pairs]))
    bms = []
    for cg in range(CGROUPS):
        Lc = LT[:, cg * C:(cg + 1) * C]                  # [edges, C]
        bm = np.stack([
            np.ascontiguousarray(
                (Lc == float(lv)).reshape(ECHUNKS, 128, C // 512, 512)
                .transpose(2, 1, 0, 3)
                .reshape(C // 512, 128, ECHUNKS * 512)).astype(NP_FP8)
            for _tv, lv in pairs])
        nulc = np.ascontiguousarray(
            null_count[cg * C:(cg + 1) * C]
            .reshape(NTILES * JTILES, 128).T).astype(np.float32)
        bms.append((np.ascontiguousarray(bm), nulc))

    in_maps = []
    for pg in range(PGROUPS):
        for cg in range(CGROUPS):
            in_maps.append({
                "Am": ams[pg],
                "Bm": bms[cg][0],
                "nulc": bms[cg][1],
            })
    return in_maps


def _kept_pairs(edge_type_filter):
    seen = []
    for v in np.asarray(edge_type_filter).ravel().tolist():
        v = int(v)
        if v in _CODE2TEMP and v not in [p[1] for p in seen]:
            seen.append((_CODE2TEMP[v], v))
    return tuple(seen)


def kernel(node_activations, learned_edge_states, edge_endnode_idx,
           edge_type_filter, _trace=False, _tmpdir=None):
    pairs = _kept_pairs(edge_type_filter)
    L = np.asarray(learned_edge_states, dtype=np.float32)
    if len(pairs) == 0:
        # nothing kept: energies are null_count rows broadcast
        null_count = (L == 0.0).sum(axis=1).astype(np.float32)
        en = np.broadcast_to(null_count[None, :], (N_PTS, N_CMP)).copy()
        return en - en.min()

    nc = _get_nc(len(pairs))
    in_maps = _prep_inputs(node_activations, learned_edge_states,
                           edge_endnode_idx, pairs)
    res = run_bass_kernel_spmd(nc, in_maps, core_ids=list(range(8)),
                               trace=_trace, tmpdir=_tmpdir)
    out = np.empty((N_PTS, N_CMP), dtype=np.float32)
    gmin = np.inf
    for ci in range(8):
        pg, cg = ci // CGROUPS, ci % CGROUPS
        r = res.results[ci]
        out[pg * P:(pg + 1) * P, cg * C:(cg + 1) * C] = r["en"].T
        gmin = min(gmin, float(r["mins"].min()))
    out -= np.float32(gmin)
    if _trace:
        kernel._last_results = res
    return out
